# revision 13
# baseline (speedup 1.0000x reference)
"""Trainium2 Bass kernel for masked graph-convolution interaction.

Math (reference):
    wf = node_features @ weight                              # [N, D]
    out[a,d] = sum_{i,j} adj[a,i]*mh[i,j]*mf[a,j]*wf[i,d]*wf[j,d] / ncnt[a]^2

Implementation (groups of 4 rows; superblocks of 4 groups):
    X_a[i,c] = adj[a,i] * wf[i,d]     c=(s,d), fp8e4; one [128,512] op per
               (group, ic) via stride-0 broadcast of the 4 adj columns,
               spread across DVE/GPSIMD/ACT
    Y[j,c]   = sum_i mh8[i,j]*X[i,c]  fp8 DoubleRow PE (2 MACs/cell/cyc)
    Z[j,c]   = Y[j,c] * wf[j,d]       bf16 (DVE, PSUM read)
    out[a,d] = sum_j mf'[j,a]*Z[j,c]  per superblock: 1-col stationary mf
               column, Z of 4 groups streams (FD=512); 4 s-rows land in 4
               PSUM partition quadrants (tile_position col packing)
    + correction: mh is sent centered (mh8 = fp8(mh-0.5)); the exact
      rank-1 term 0.5*(adj@wf)*(mf@wf)/ncnt^2 is computed on host and
      added during the PSUM drain. mf' has 1/ncnt^2 folded in on host.

Sharding: row-split of a across 8 cores (128 rows each); mh8/wf replicated.
"""

import numpy as np

N = 1024
DIN = 256
DOUT = 128
NCORES = 8
ROWS = N // NCORES  # 128 output rows per core
P = 128
IC = N // P  # 8 contraction chunks over i
KK = IC // 2  # 4 DoubleRow pairs
JC = N // P  # 8 chunks over j
G4 = 4  # rows per group (psum free dim 4*128 = 512)
NG = ROWS // G4  # 32 groups per core
SUP = 4  # groups per matvec superblock
NSUP = NG // SUP

# X-prep engine split by ic chunk: 'g'=gpsimd bigop, 'v'=DVE bigop, 'a'=ACT
XPREP = ["g", "g", "g", "g", "g", "a", "a", "a"]

_CACHE = {}


def _build():
    """Build + compile the Bass module (shared across all 8 cores, SPMD)."""
    import concourse.bass as bass
    import concourse.tile as tile
    from concourse import bacc, mybir
    from concourse._compat import axon_active

    f32 = mybir.dt.float32
    dt8 = mybir.dt.float8e4
    dtb = mybir.dt.bfloat16
    Copy = mybir.ActivationFunctionType.Copy
    DR = mybir.MatmulPerfMode.DoubleRow

    nc = bacc.Bacc(
        "TRN2",
        target_bir_lowering=False,
        debug=not axon_active(),
        num_devices=NCORES,
    )

    mh_d = nc.dram_tensor("mh8", [N, N], dt8, kind="ExternalInput").ap()
    wf_d = nc.dram_tensor("wf", [N, DOUT], dtb, kind="ExternalInput").ap()
    adjTb_d = nc.dram_tensor("adjTb", [5 * P, ROWS], dtb, kind="ExternalInput").ap()
    adjTf_d = nc.dram_tensor("adjTf", [3 * P, ROWS], f32, kind="ExternalInput").ap()
    mfT_d = nc.dram_tensor("mfT", [N, ROWS], dtb, kind="ExternalInput").ap()
    out_d = nc.dram_tensor("out", [ROWS, DOUT], f32, kind="ExternalOutput").ap()

    with tile.TileContext(nc) as tc:
        with (
            tc.tile_pool(name="const", bufs=1) as cpool,
            tc.tile_pool(name="x", bufs=4) as xpool,
            tc.tile_pool(name="z", bufs=20) as zpool,
            tc.tile_pool(name="py", bufs=6, space="PSUM") as pypool,
            tc.tile_pool(name="mv", bufs=2, space="PSUM") as mvpool,
        ):
            # ---- input DMA across both HWDGE queues, critical-path first:
            # sync:   wf/adjT halves interleaved, then mh pair 3
            # scalar: mh pairs 0-2, mfT, fin
            wf_sb = cpool.tile([P, N], dtb, tag="wf")
            adjTb_sb = cpool.tile([P, 5 * P], dtb, tag="adjTb")
            adjTf_sb = cpool.tile([P, 3 * P], f32, tag="adjTf")
            mh_sb = cpool.tile([P, IC, N], dt8, tag="mh")
            mfT_sb = cpool.tile([P, N], dtb, tag="mfT")

            def half(src_d, dst_sb, k):
                nc.sync.dma_start(
                    dst_sb[:, k * 512 : (k + 1) * 512].rearrange(
                        "p (c d) -> p c d", c=4
                    ),
                    src_d[k * 512 : (k + 1) * 512, :].rearrange(
                        "(c p) d -> p c d", p=P
                    ),
                )

            def mh_pair(eng, k):
                eng.dma_start(
                    mh_sb[:, 2 * k : 2 * k + 2, :],
                    mh_d[2 * k * P : (2 * k + 2) * P, :].rearrange(
                        "(e p) j -> p e j", p=P
                    ),
                )

            mh_pair(nc.scalar, 0)
            half(wf_d, wf_sb, 0)
            nc.sync.dma_start(
                adjTb_sb[:].rearrange("p (c a) -> p c a", c=5),
                adjTb_d[:].rearrange("(c p) a -> p c a", p=P),
            )
            mh_pair(nc.scalar, 1)
            half(wf_d, wf_sb, 1)
            nc.sync.dma_start(
                adjTf_sb[:].rearrange("p (c a) -> p c a", c=3),
                adjTf_d[:].rearrange("(c p) a -> p c a", p=P),
            )
            mh_pair(nc.sync, 3)
            mh_pair(nc.scalar, 2)
            nc.scalar.dma_start(
                mfT_sb[:].rearrange("p (c a) -> p c a", c=8),
                mfT_d[:].rearrange("(c p) a -> p c a", p=P),
            )

            out_pack = cpool.tile([P, NG * 512], f32, tag="opack")

            # ---- main loop (stage-2 of group g emitted after stage-1 of
            # group g+1 so the PE never head-of-line blocks on the last
            # Z-mult of a group) ----
            mv_tiles = {}
            z_of = {}

            def stage2(g):
                # out[s', (s,d)] = sum_j mf'[j, a_s'] * Z[j, (s,d)]; the
                # diagonal blocks s'==s are the 4 row results. Plain MMs:
                # LDW (4 cols) is background-buffer eligible, so stage-2
                # pipelines with the DR stream with no array drains.
                mvt = mvpool.tile([P, 512], f32, tag="mv", name=f"mv{g}")
                for jc in range(JC):
                    nc.tensor.matmul(
                        mvt[0:G4, :],
                        lhsT=mfT_sb[:, jc * P + g * G4 : jc * P + (g + 1) * G4],
                        rhs=z_of[g][jc][:],
                        start=(jc == 0),
                        stop=(jc == JC - 1),
                    )
                del z_of[g]
                nc.vector.tensor_copy(
                    out_pack[0:G4, g * 512 : (g + 1) * 512], mvt[0:G4, :]
                )
                if g % SUP == SUP - 1:
                    gsup = g // SUP
                    for s in range(G4):
                        nc.sync.dma_start(
                            out_d[
                                gsup * SUP * G4 + s : gsup * SUP * G4
                                + s + 4 * (SUP - 1) + 1 : 4,
                                :,
                            ],
                            out_pack[
                                s : s + 1,
                                gsup * SUP * 512 : (gsup + 1) * SUP * 512,
                            ].rearrange("p (g y) -> p g y", g=SUP)[
                                :, :, s * DOUT : (s + 1) * DOUT
                            ],
                        )

            for b in range(NG):

                # X[(ic), c]: X_a[i,(s,d)] = adj[a,i] * wf[i,d] -> fp8
                x_t = xpool.tile([P, IC, 512], dt8, tag="X")
                for ic in range(IC):
                    kind = XPREP[ic]
                    if kind in ("g", "v"):
                        ab = (
                            adjTb_sb[:, ic * P + b * G4 : ic * P + (b + 1) * G4]
                            .unsqueeze(-1)
                            .broadcast_to([P, G4, DOUT])
                        )
                        wv = (
                            wf_sb[:, ic * DOUT : (ic + 1) * DOUT]
                            .unsqueeze(1)
                            .broadcast_to([P, G4, DOUT])
                        )
                        xv = x_t[:, ic, :].rearrange("p (s d) -> p s d", s=G4)
                        eng = nc.gpsimd if kind == "g" else nc.vector
                        eng.tensor_mul(xv, ab, wv)
                    else:
                        for s in range(G4):
                            a = b * G4 + s
                            nc.scalar.activation(
                                x_t[:, ic, s * DOUT : (s + 1) * DOUT],
                                wf_sb[:, ic * DOUT : (ic + 1) * DOUT],
                                Copy,
                                scale=adjTf_sb[:, (ic - 5) * P + a : (ic - 5) * P + a + 1],
                            )

                # stage 1: Y = mh8^T X (DoubleRow), Z = Y * wf
                z_ts = []
                for jc in range(JC):
                    py = pypool.tile([P, 512], f32, tag="py")
                    for kk in range(KK):
                        nc.tensor.matmul(
                            py[:],
                            lhsT=mh_sb[:, 2 * kk : 2 * kk + 2, jc * P : (jc + 1) * P],
                            rhs=x_t[:, 2 * kk : 2 * kk + 2, :],
                            start=(kk == 0),
                            stop=(kk == KK - 1),
                            perf_mode=DR,
                        )
                    z_t = zpool.tile([P, 512], dtb, tag="Z")
                    nc.vector.tensor_mul(
                        z_t[:].rearrange("p (s d) -> p s d", s=G4),
                        py[:].rearrange("p (s d) -> p s d", s=G4),
                        wf_sb[:, jc * DOUT : (jc + 1) * DOUT]
                        .unsqueeze(1)
                        .broadcast_to([P, G4, DOUT]),
                    )
                    z_ts.append(z_t)
                z_of[b] = z_ts
                stage2(b)

    nc.compile()
    return nc


def _prep_inputs(inputs):
    """Host-side sharding + layout prep. Returns per-core input maps."""
    import ml_dtypes

    f8 = ml_dtypes.float8_e4m3fn
    bf16 = ml_dtypes.bfloat16

    nf = np.asarray(inputs["node_features"], dtype=np.float64)
    adj = np.asarray(inputs["adjacency_matrix"], dtype=np.float32)
    mf = np.asarray(inputs["mask_father"], dtype=np.float32)[:, 0, :]
    ncnt = np.asarray(inputs["neighbor_count"], dtype=np.float32)
    mh = np.asarray(inputs["mask_hadamard"], dtype=np.float32)
    w = np.asarray(inputs["weight"], dtype=np.float64)

    wf = (nf @ w).astype(np.float32)  # [N, D]
    mh8 = np.ascontiguousarray(
        (mh[:, 0, :].astype(np.float64) - 0.5).astype(np.float32)
    ).astype(f8)
    inv2 = 1.0 / (ncnt.astype(np.float64) ** 2)  # [N,1]

    wf64 = wf.astype(np.float64)
    S_all = adj.astype(np.float64) @ wf64  # [N, D]
    V_all = mf.astype(np.float64) @ wf64  # [N, D]
    fin_all = (0.5 * S_all * V_all * inv2).astype(np.float32)  # [N, D]

    mfs = mf.astype(np.float64) * inv2  # mf with 1/ncnt^2 folded in

    in_maps = []
    for c in range(NCORES):
        rows = slice(c * ROWS, (c + 1) * ROWS)
        in_maps.append(
            {
                "mh8": mh8,
                "wf": wf.astype(bf16),
                "adjTb": np.ascontiguousarray(adj[rows].T[: 5 * P]).astype(bf16),
                "adjTf": np.ascontiguousarray(adj[rows].T[5 * P :]),
                "mfT": np.ascontiguousarray(mfs[rows].T.astype(np.float32)).astype(
                    bf16
                ),
            }
        )
    return in_maps, fin_all


def _run(inputs, trace=False):
    from concourse import bass_utils

    if "nc" not in _CACHE:
        _CACHE["nc"] = _build()
    nc = _CACHE["nc"]
    in_maps, fin_all = _prep_inputs(inputs)
    res = bass_utils.run_bass_kernel_spmd(
        nc, in_maps, core_ids=list(range(NCORES)), trace=trace
    )
    out = np.concatenate([r["out"] for r in res.results], axis=0) + fin_all
    return out.astype(np.float32), res


def kernel(**inputs):
    out, _ = _run(inputs, trace=False)
    return out


# revision 14
# speedup vs baseline: 1.1430x; 1.1430x over previous
"""Trainium2 Bass kernel for masked graph-convolution interaction.

Math (reference):
    wf = node_features @ weight                              # [N, D]
    out[a,d] = sum_{i,j} adj[a,i]*mh[i,j]*mf[a,j]*wf[i,d]*wf[j,d] / ncnt[a]^2

Implementation (groups of 4 rows; superblocks of 4 groups):
    X_a[i,c] = adj[a,i] * wf[i,d]     c=(s,d), fp8e4; one [128,512] op per
               (group, ic) via stride-0 broadcast of the 4 adj columns,
               spread across DVE/GPSIMD/ACT
    Y[j,c]   = sum_i mh8[i,j]*X[i,c]  fp8 DoubleRow PE (2 MACs/cell/cyc)
    Z[j,c]   = Y[j,c] * wf[j,d]       bf16 (DVE, PSUM read)
    out[a,d] = sum_j mf'[j,a]*Z[j,c]  per superblock: 1-col stationary mf
               column, Z of 4 groups streams (FD=512); 4 s-rows land in 4
               PSUM partition quadrants (tile_position col packing)
    + correction: mh is sent centered (mh8 = fp8(mh-0.5)); the exact
      rank-1 term 0.5*(adj@wf)*(mf@wf)/ncnt^2 is computed on host and
      added during the PSUM drain. mf' has 1/ncnt^2 folded in on host.

Sharding: row-split of a across 8 cores (128 rows each); mh8/wf replicated.
"""

import numpy as np

N = 1024
DIN = 256
DOUT = 128
NCORES = 8
ROWS = N // NCORES  # 128 output rows per core
P = 128
IC = N // P  # 8 contraction chunks over i
KK = IC // 2  # 4 DoubleRow pairs
JC = N // P  # 8 chunks over j
G4 = 4  # rows per group (psum free dim 4*128 = 512)
NG = ROWS // G4  # 32 groups per core
SUP = 4  # groups per matvec superblock
NSUP = NG // SUP

# X-prep engine split by ic chunk: 'g'=gpsimd bigop, 'v'=DVE bigop, 'a'=ACT
XPREP = ["g", "g", "g", "g", "g", "a", "a", "a"]

_CACHE = {}


def _build():
    """Build + compile the Bass module (shared across all 8 cores, SPMD)."""
    import concourse.bass as bass
    import concourse.tile as tile
    from concourse import bacc, mybir
    from concourse._compat import axon_active

    f32 = mybir.dt.float32
    dt8 = mybir.dt.float8e4
    dtb = mybir.dt.bfloat16
    Copy = mybir.ActivationFunctionType.Copy
    DR = mybir.MatmulPerfMode.DoubleRow

    nc = bacc.Bacc(
        "TRN2",
        target_bir_lowering=False,
        debug=not axon_active(),
        num_devices=NCORES,
    )

    mh_d = nc.dram_tensor("mh8", [N, N], dt8, kind="ExternalInput").ap()
    wf_d = nc.dram_tensor("wf", [N, DOUT], dtb, kind="ExternalInput").ap()
    adjTb_d = nc.dram_tensor("adjTb", [5 * P, ROWS], dtb, kind="ExternalInput").ap()
    adjTf_d = nc.dram_tensor("adjTf", [3 * P, ROWS], f32, kind="ExternalInput").ap()
    mfT_d = nc.dram_tensor("mfT", [N, ROWS], dtb, kind="ExternalInput").ap()
    out_d = nc.dram_tensor("out", [ROWS, DOUT], f32, kind="ExternalOutput").ap()

    with tile.TileContext(nc) as tc:
        with (
            tc.tile_pool(name="const", bufs=1) as cpool,
            tc.tile_pool(name="x", bufs=4) as xpool,
            tc.tile_pool(name="z", bufs=20) as zpool,
            tc.tile_pool(name="py", bufs=6, space="PSUM") as pypool,
            tc.tile_pool(name="mv", bufs=2, space="PSUM") as mvpool,
        ):
            # ---- input DMA across both HWDGE queues, critical-path first:
            # sync:   wf/adjT halves interleaved, then mh pair 3
            # scalar: mh pairs 0-2, mfT, fin
            wf_sb = cpool.tile([P, N], dtb, tag="wf")
            adjTb_sb = cpool.tile([P, 5 * P], dtb, tag="adjTb")
            adjTf_sb = cpool.tile([P, 3 * P], f32, tag="adjTf")
            mh_sb = cpool.tile([P, IC, N], dt8, tag="mh")
            mfT_sb = cpool.tile([P, N], dtb, tag="mfT")

            def half(src_d, dst_sb, k):
                nc.sync.dma_start(
                    dst_sb[:, k * 512 : (k + 1) * 512].rearrange(
                        "p (c d) -> p c d", c=4
                    ),
                    src_d[k * 512 : (k + 1) * 512, :].rearrange(
                        "(c p) d -> p c d", p=P
                    ),
                )

            def mh_pair(eng, k):
                eng.dma_start(
                    mh_sb[:, 2 * k : 2 * k + 2, :],
                    mh_d[2 * k * P : (2 * k + 2) * P, :].rearrange(
                        "(e p) j -> p e j", p=P
                    ),
                )

            mh_pair(nc.scalar, 0)
            half(wf_d, wf_sb, 0)
            nc.sync.dma_start(
                adjTb_sb[:].rearrange("p (c a) -> p c a", c=5),
                adjTb_d[:].rearrange("(c p) a -> p c a", p=P),
            )
            mh_pair(nc.scalar, 1)
            half(wf_d, wf_sb, 1)
            nc.sync.dma_start(
                adjTf_sb[:].rearrange("p (c a) -> p c a", c=3),
                adjTf_d[:].rearrange("(c p) a -> p c a", p=P),
            )
            mh_pair(nc.sync, 3)
            mh_pair(nc.scalar, 2)
            nc.scalar.dma_start(
                mfT_sb[:].rearrange("p (c a) -> p c a", c=8),
                mfT_d[:].rearrange("(c p) a -> p c a", p=P),
            )

            out_pack = cpool.tile([P, NG * 512], f32, tag="opack")

            # ---- main loop (stage-2 of group g emitted after stage-1 of
            # group g+1 so the PE never head-of-line blocks on the last
            # Z-mult of a group) ----
            mv_tiles = {}
            z_of = {}

            def stage2(g):
                # out[s', (s,d)] = sum_j mf'[j, a_s'] * Z[j, (s,d)]; the
                # diagonal blocks s'==s are the 4 row results. Plain MMs:
                # LDW (4 cols) is background-buffer eligible, so stage-2
                # pipelines with the DR stream with no array drains.
                mvt = mvpool.tile([P, 512], f32, tag="mv", name=f"mv{g}")
                # jc=7 first: its Z is the last ready, so all 8 MMs become
                # schedulable together and run contiguously -> only one
                # DoubleRow<->normal mode switch per group each way
                order = [JC - 1] + list(range(JC - 1))
                for k, jc in enumerate(order):
                    nc.tensor.matmul(
                        mvt[0:G4, :],
                        lhsT=mfT_sb[:, jc * P + g * G4 : jc * P + (g + 1) * G4],
                        rhs=z_of[g][jc][:],
                        start=(k == 0),
                        stop=(k == JC - 1),
                    )
                del z_of[g]
                nc.vector.tensor_copy(
                    out_pack[0:G4, g * 512 : (g + 1) * 512], mvt[0:G4, :]
                )
                if g % SUP == SUP - 1:
                    gsup = g // SUP
                    for s in range(G4):
                        nc.sync.dma_start(
                            out_d[
                                gsup * SUP * G4 + s : gsup * SUP * G4
                                + s + 4 * (SUP - 1) + 1 : 4,
                                :,
                            ],
                            out_pack[
                                s : s + 1,
                                gsup * SUP * 512 : (gsup + 1) * SUP * 512,
                            ].rearrange("p (g y) -> p g y", g=SUP)[
                                :, :, s * DOUT : (s + 1) * DOUT
                            ],
                        )

            for b in range(NG):

                # X[(ic), c]: X_a[i,(s,d)] = adj[a,i] * wf[i,d] -> fp8
                x_t = xpool.tile([P, IC, 512], dt8, tag="X")
                for ic in range(IC):
                    kind = XPREP[ic]
                    if kind in ("g", "v"):
                        ab = (
                            adjTb_sb[:, ic * P + b * G4 : ic * P + (b + 1) * G4]
                            .unsqueeze(-1)
                            .broadcast_to([P, G4, DOUT])
                        )
                        wv = (
                            wf_sb[:, ic * DOUT : (ic + 1) * DOUT]
                            .unsqueeze(1)
                            .broadcast_to([P, G4, DOUT])
                        )
                        xv = x_t[:, ic, :].rearrange("p (s d) -> p s d", s=G4)
                        eng = nc.gpsimd if kind == "g" else nc.vector
                        eng.tensor_mul(xv, ab, wv)
                    else:
                        for s in range(G4):
                            a = b * G4 + s
                            nc.scalar.activation(
                                x_t[:, ic, s * DOUT : (s + 1) * DOUT],
                                wf_sb[:, ic * DOUT : (ic + 1) * DOUT],
                                Copy,
                                scale=adjTf_sb[:, (ic - 5) * P + a : (ic - 5) * P + a + 1],
                            )

                # stage 1: Y = mh8^T X (DoubleRow), Z = Y * wf
                z_ts = []
                for jc in range(JC):
                    py = pypool.tile([P, 512], f32, tag="py")
                    for kk in range(KK):
                        nc.tensor.matmul(
                            py[:],
                            lhsT=mh_sb[:, 2 * kk : 2 * kk + 2, jc * P : (jc + 1) * P],
                            rhs=x_t[:, 2 * kk : 2 * kk + 2, :],
                            start=(kk == 0),
                            stop=(kk == KK - 1),
                            perf_mode=DR,
                        )
                    z_t = zpool.tile([P, 512], dtb, tag="Z")
                    nc.vector.tensor_mul(
                        z_t[:].rearrange("p (s d) -> p s d", s=G4),
                        py[:].rearrange("p (s d) -> p s d", s=G4),
                        wf_sb[:, jc * DOUT : (jc + 1) * DOUT]
                        .unsqueeze(1)
                        .broadcast_to([P, G4, DOUT]),
                    )
                    z_ts.append(z_t)
                z_of[b] = z_ts
                stage2(b)

    nc.compile()
    return nc


def _prep_inputs(inputs):
    """Host-side sharding + layout prep. Returns per-core input maps."""
    import ml_dtypes

    f8 = ml_dtypes.float8_e4m3fn
    bf16 = ml_dtypes.bfloat16

    nf = np.asarray(inputs["node_features"], dtype=np.float64)
    adj = np.asarray(inputs["adjacency_matrix"], dtype=np.float32)
    mf = np.asarray(inputs["mask_father"], dtype=np.float32)[:, 0, :]
    ncnt = np.asarray(inputs["neighbor_count"], dtype=np.float32)
    mh = np.asarray(inputs["mask_hadamard"], dtype=np.float32)
    w = np.asarray(inputs["weight"], dtype=np.float64)

    wf = (nf @ w).astype(np.float32)  # [N, D]
    mh8 = np.ascontiguousarray(
        (mh[:, 0, :].astype(np.float64) - 0.5).astype(np.float32)
    ).astype(f8)
    inv2 = 1.0 / (ncnt.astype(np.float64) ** 2)  # [N,1]

    wf64 = wf.astype(np.float64)
    S_all = adj.astype(np.float64) @ wf64  # [N, D]
    V_all = mf.astype(np.float64) @ wf64  # [N, D]
    fin_all = (0.5 * S_all * V_all * inv2).astype(np.float32)  # [N, D]

    mfs = mf.astype(np.float64) * inv2  # mf with 1/ncnt^2 folded in

    in_maps = []
    for c in range(NCORES):
        rows = slice(c * ROWS, (c + 1) * ROWS)
        in_maps.append(
            {
                "mh8": mh8,
                "wf": wf.astype(bf16),
                "adjTb": np.ascontiguousarray(adj[rows].T[: 5 * P]).astype(bf16),
                "adjTf": np.ascontiguousarray(adj[rows].T[5 * P :]),
                "mfT": np.ascontiguousarray(mfs[rows].T.astype(np.float32)).astype(
                    bf16
                ),
            }
        )
    return in_maps, fin_all


def _run(inputs, trace=False):
    from concourse import bass_utils

    if "nc" not in _CACHE:
        _CACHE["nc"] = _build()
    nc = _CACHE["nc"]
    in_maps, fin_all = _prep_inputs(inputs)
    res = bass_utils.run_bass_kernel_spmd(
        nc, in_maps, core_ids=list(range(NCORES)), trace=trace
    )
    out = np.concatenate([r["out"] for r in res.results], axis=0) + fin_all
    return out.astype(np.float32), res


def kernel(**inputs):
    out, _ = _run(inputs, trace=False)
    return out
